# revision 26
# baseline (speedup 1.0000x reference)
"""Trainium2 Bass kernel for channel attention (XCA-style) module.

Computation (per batch b):
  qkv = w_qkv @ x          (1x1 conv, 192 -> 576 ch)
  qkv = dwconv3x3(qkv)     (depthwise, pad 1)
  q,k,v = split; per head (48 ch): l2-normalize q,k along spatial,
  attn = softmax(temp * q_hat k_hat^T); out = attn @ v
  out = w_proj @ out       (1x1 conv)

Sharding: 8 cores = 4 batches x 2 head-pairs. Each core handles one batch and
2 of the 4 heads (288 of 576 qkv channels), producing a partial projection
output [192, 16384] in bf16; host sums the two partials per batch.

v2 structure (per core):
  Phase 1 (sweep A): x streamed once; per 1024-px chunk-pair, GEMMs for all
    3 groups into [128,1024] 2-bank PSUM tiles; paired evacuation to padded
    image buffers: q/k in fp8e4 (row stride 144), v in bf16 (row stride 132).
  Phase 2 (q,k): depthwise taps fully on PE as fp8 matmuls; the (dy=-1,dy=0)
    taps per dx are fused into one DoubleRow pair matmul (2 taps/instruction);
    dy=+1 taps are plain fp8 matmuls. Evac -> bf16 -> PE transpose -> qkT
    store (DVE 2x copy) -> per-head gram accumulation.
  Phase 3 (v): depthwise taps fully on DVE as 3 accumulation chains over
    16-row flat spans (chain starts are 4x-mode tensor_scalar on the aligned
    dx=0 taps); two tensor_tensor merges; the final projection GEMM reads the
    merged accumulator directly (no copy), output evacuated as bf16.
"""

import sys

sys.path.insert(0, "/opt/trn_rl_repo")

import numpy as np
import ml_dtypes

import concourse.bass as bass
import concourse.mybir as mybir
from concourse import tile
from concourse.bass_utils import run_bass_kernel_spmd

F32 = mybir.dt.float32
BF16 = mybir.dt.bfloat16
F8 = mybir.dt.float8e4

DIM = 192
HEADS = 4
B = 4
HH = 128
WW = 128
NPIX = HH * WW          # 16384
GCH = 96                # q/k/v channels per core (2 heads x 48)
NPAIR = 16              # 1024-px chunk pairs
PCH = 1024              # pair chunk size (8 image rows)
RS8 = 144               # fp8 pad row stride (8 pad cols each side)
C08 = 8                 # fp8 pad interior col offset
RSV = 132               # bf16 v pad row stride (2 pad cols each side)
VSPAN = 16              # v-tap span in image rows (4 chunks of 512)
NVSP = HH // VSPAN      # 8 spans
EPS = 1e-12


def _split_multiwait(nc):
    """walrus in this env only encodes one sem-wait per instruction; hoist
    extra waits into single-wait NoOps placed just before the instruction."""
    for f in nc.m.functions:
        for bb in f.blocks:
            insts = bb.instructions
            i = 0
            while i < len(insts):
                inst = insts[i]
                si = getattr(inst, "sync_info", None)
                ow = list(si.on_wait) if (si is not None and si.on_wait) else []
                if len(ow) > 1:
                    nops = []
                    for w in ow[:-1]:
                        nops.append(
                            mybir.InstNoOp(
                                name=nc.get_next_instruction_name(),
                                sync_info=mybir.SyncInfo(on_wait=[w], on_update=[]),
                                bass_nofuse=True,
                                engine=inst.engine,
                            )
                        )
                    inst.sync_info = mybir.SyncInfo(
                        on_wait=[ow[-1]], on_update=list(si.on_update)
                    )
                    insts[i:i] = nops
                    i += len(nops)
                i += 1


def _pair_window(pad8, nrows_buf, row0, c0, w=128):
    """Overlapping [128, 2, 4, w] AP over an fp8 pad: two 4-row windows at
    buffer rows row0 and row0+1 (the dy=-1 / dy=0 tap pair), cols c0..c0+w.
    Pair stride RS8 satisfies the DoubleRow step%16==0 constraint."""
    r = pad8.rearrange("p (r x) -> p r x", r=nrows_buf, x=RS8)
    base = r[:, row0 : row0 + 5, c0 : c0 + w]          # [p, 5, w]
    u = base.unsqueeze(1).broadcast_to((128, 2, 5, w))  # [p, 2, 5, w] stride 0
    apv = u.ap
    apv[1] = [RS8, 2]
    c = u.copy()
    c.ap = apv
    return c[:, :, 0:4, :]


def _build_kernel():
    nc = bass.Bass("TRN2", target_bir_lowering=False, debug=False, num_devices=8)

    # ---- DRAM I/O ----
    x_half = [
        nc.dram_tensor(f"x{h}", [GCH, NPIX], BF16, kind="ExternalInput")
        for h in range(2)
    ]  # input channels 0:96 / 96:192, bf16
    # w_qkv^T column layout: 3 groups x 128-padded [h0(48) pad h1(48) pad]
    wqT0 = nc.dram_tensor("wqT0", [GCH, 384], BF16, kind="ExternalInput")
    wqT1 = nc.dram_tensor("wqT1", [GCH, 384], BF16, kind="ExternalInput")
    # fp8 diag tap weights for q/k: per group, 3 DoubleRow pairs (dy=-1,0) at
    # dx=-1,0,1 then 3 singles (dy=+1): [128, 2*(3*256+3*128)] = [128, 2304]
    wd8 = nc.dram_tensor("wd8", [128, 2304], F8, kind="ExternalInput")
    wpT = nc.dram_tensor("wpT", [GCH, DIM], BF16, kind="ExternalInput")
    wtap = nc.dram_tensor("wtap", [128, 27], F32, kind="ExternalInput")
    tempv = nc.dram_tensor("tempv", [48, 2], F32, kind="ExternalInput")
    id128 = nc.dram_tensor("id128", [128, 128], F32, kind="ExternalInput")
    outp = nc.dram_tensor("outp", [DIM, NPIX], BF16, kind="ExternalOutput")

    GLO = [0, 128, 256]    # group column offsets in wqT
    W = 128                # padded group width (h0 at 0:48, h1 at 64:112)

    with tile.TileContext(nc) as tc:
        with (
            tc.tile_pool(name="persist", bufs=1) as pp,
            tc.tile_pool(name="scratch", bufs=2) as sp,
            tc.tile_pool(name="dsc", bufs=3) as dscp,
            tc.tile_pool(name="ost", bufs=6) as ostp,
            tc.tile_pool(name="acc", bufs=2) as accp,
            tc.tile_pool(name="xstream", bufs=4) as xsp,
            tc.tile_pool(name="ps_big", bufs=3, space="PSUM") as psb,
            tc.tile_pool(name="ps_tr", bufs=1, space="PSUM") as pst,
            tc.tile_pool(name="ps_gram", bufs=1, space="PSUM") as psg,
        ):
            # ---- persistent SBUF ----
            w0 = pp.tile([GCH, 384], BF16, tag="w0")
            w1 = pp.tile([GCH, 384], BF16, tag="w1")
            nc.sync.dma_start(w0[:], wqT0[:])
            nc.sync.dma_start(w1[:], wqT1[:])
            xpre = []
            for pp_i in range(3):
                xa = xsp.tile([GCH, PCH], BF16, tag="xs0", name=f"xpre0_{pp_i}")
                xb = xsp.tile([GCH, PCH], BF16, tag="xs1", name=f"xpre1_{pp_i}")
                nc.sync.dma_start(xa[:], x_half[0][:, PCH * pp_i : PCH * (pp_i + 1)])
                nc.sync.dma_start(xb[:], x_half[1][:, PCH * pp_i : PCH * (pp_i + 1)])
                xpre.append((xa, xb))
            wd8t = pp.tile([128, 2304], F8, tag="wd8t")
            nc.sync.dma_start(wd8t[:], wd8[:])
            wpb = pp.tile([112, DIM], BF16, tag="wpb")
            for h in range(2):
                nc.sync.dma_start(wpb[64 * h : 64 * h + 48, :],
                                  wpT[48 * h : 48 * h + 48, :])
            wt = pp.tile([128, 27], F32, tag="wt")
            nc.sync.dma_start(wt[:], wtap[:])
            tv = pp.tile([48, 2], F32, tag="tv")
            nc.sync.dma_start(tv[:], tempv[:])
            tvb = pp.tile([112, 1], F32, tag="tvb")
            nc.vector.tensor_copy(tvb[0:48, :], tv[:, 0:1])
            nc.vector.tensor_copy(tvb[64:112, :], tv[:, 1:2])
            idf = pp.tile([128, 128], F32, tag="idf")
            nc.sync.dma_start(idf[:], id128[:])
            idb = pp.tile([128, 128], BF16, tag="idb")
            nc.vector.tensor_copy(idb[:], idf[:])
            # preload ACT table sets (exp, sqrt) so attn finalize pays no
            # table-load latency on the critical path
            warm = pp.tile([1, 1], F32, tag="warm")
            nc.scalar.activation(warm[:], idf[0:1, 0:1],
                                 mybir.ActivationFunctionType.Exp)
            nc.scalar.sqrt(warm[:], idf[0:1, 0:1])

            # fp8 padded image buffers for q/k: 130 rows x RS8 cols
            pad8s = []
            for gi in range(2):
                p8 = pp.tile([128, 130 * RS8], F8, tag=f"pad8_{gi}",
                             name=f"pad8_{gi}")
                r = p8[:].rearrange("p (r x) -> p r x", r=130, x=RS8)
                nc.vector.memset(r[:, 0, :], 0.0)
                nc.vector.memset(r[:, 129, :], 0.0)
                nc.scalar.memzero(r[:, :, 0:C08])
                nc.scalar.memzero(r[:, :, C08 + 128 :])
                pad8s.append(p8)

            # bf16 padded image buffer for v: 130 rows x RSV cols + tail
            padvf = pp.tile([128, 130 * RSV + 4], BF16, tag="padv")
            padv = padvf[:, 0 : 130 * RSV].rearrange(
                "p (r x) -> p r x", r=130, x=RSV
            )
            nc.vector.memset(padv[:, 0, :], 0.0)
            nc.vector.memset(padv[:, 129, :], 0.0)
            nc.vector.memset(padv[:, :, 0:2], 0.0)
            nc.vector.memset(padv[:, :, 130:132], 0.0)
            nc.vector.memset(padvf[:, 130 * RSV :], 0.0)

            # transposed q/k store: per 128-pixel subtile s, 256 columns:
            # [q_h0(48) pad(16) k_h0(48) pad(16) q_h1(48) pad(16) k_h1(48) pad(16)]
            qkT = pp.tile([128, 128 * 256], F8, tag="qkT")
            qkTr = qkT[:].rearrange("p (s b c) -> p s b c", s=128, b=2, c=128)
            qkTp = qkT[:].rearrange("p (s b c) -> p s b c", s=128, b=4, c=64)
            nc.vector.memset(qkTp[:, :, :, 48:64], 0.0)

            # gram accumulators: both heads in one PSUM bank, [128, 2*128]
            gram = psg.tile([128, 256], F32, tag="gram")

            # M^T for final GEMM: [128(d-padded), 192(o)] bf16
            mt = pp.tile([128, DIM], BF16, tag="mt")
            nc.vector.memset(mt[:], 0.0)

            # persistent v-dwconv result slab (holds all spans until mt ready)
            vdw = pp.tile([128, NVSP * VSPAN * RSV], BF16, tag="vdw")

            # ========== fused pipeline: sweepA(p) | taps(p-1) | chains ==========
            def sweep_a(p):
                if p < 3:
                    xt0, xt1 = xpre[p]
                else:
                    xt0 = xsp.tile([GCH, PCH], BF16, tag="xs0")
                    xt1 = xsp.tile([GCH, PCH], BF16, tag="xs1")
                    nc.sync.dma_start(xt0[:], x_half[0][:, PCH * p : PCH * (p + 1)])
                    nc.sync.dma_start(xt1[:], x_half[1][:, PCH * p : PCH * (p + 1)])
                for g in range(3):
                    praw = psb.tile([128, PCH], F32, tag="big",
                                    name=f"praw_{p}_{g}")
                    for h, (wh, xt) in enumerate(((w0, xt0), (w1, xt1))):
                        for hf in range(2):
                            nc.tensor.matmul(
                                praw[:, 512 * hf : 512 * hf + 512],
                                wh[:, GLO[g] : GLO[g] + W],
                                xt[:, 512 * hf : 512 * hf + 512],
                                start=(h == 0), stop=(h == 1),
                                skip_group_check=True,
                            )
                    src_v = praw[:].rearrange("p (r x) -> p r x", r=8, x=128)
                    if g < 2:
                        dst = pad8s[g][:].rearrange(
                            "p (r x) -> p r x", r=130, x=RS8
                        )[:, 8 * p + 1 : 8 * p + 9, C08 : C08 + 128]
                        nc.scalar.copy(dst, src_v)
                    else:
                        dst = padv[:, 8 * p + 1 : 8 * p + 9, 2:130]
                        if p < 2:
                            nc.vector.tensor_copy(dst, src_v)
                        else:
                            nc.scalar.copy(dst, src_v)

            def taps_pair(p):
                for g in range(2):
                    pad8 = pad8s[g][:]
                    pdw = psb.tile([128, PCH], F32, tag="big",
                                   name=f"pdw_{g}_{p}")
                    for ci, dx in enumerate((-1, 0, 1)):
                        wpair = wd8t[:, 1152 * g + 256 * ci :
                                     1152 * g + 256 * ci + 256].rearrange(
                            "p (t x) -> p t x", t=2, x=128
                        )
                        for hf in range(2):
                            r0 = 8 * p + 4 * hf
                            rhs = _pair_window(pad8, 130, r0, C08 + dx)
                            nc.tensor.matmul(
                                pdw[:, 512 * hf : 512 * hf + 512],
                                wpair, rhs,
                                start=(ci == 0), stop=False,
                                perf_mode=mybir.MatmulPerfMode.DoubleRow,
                                skip_group_check=True,
                            )
                    r3 = pad8.rearrange("p (r x) -> p r x", r=130, x=RS8)
                    for ci, dx in enumerate((-1, 0, 1)):
                        ws = wd8t[:, 1152 * g + 768 + 128 * ci :
                                  1152 * g + 768 + 128 * ci + 128]
                        for hf in range(2):
                            r0 = 8 * p + 4 * hf
                            rhs = r3[:, r0 + 2 : r0 + 6,
                                     C08 + dx : C08 + dx + 128]
                            nc.tensor.matmul(
                                pdw[:, 512 * hf : 512 * hf + 512],
                                ws, rhs,
                                start=False, stop=(ci == 2),
                                skip_group_check=True,
                            )
                    dsc = dscp.tile([128, PCH], BF16, tag="dsc",
                                    name=f"dsc_{g}_{p}")
                    nc.scalar.copy(dsc[:], pdw[:])
                    ptr = pst.tile([128, PCH], BF16, tag="ptr",
                                   name=f"ptr_{g}_{p}")
                    for j in range(8):
                        nc.tensor.transpose(
                            ptr[:, 128 * j : 128 * (j + 1)],
                            dsc[:, 128 * j : 128 * (j + 1)],
                            idb[:],
                        )
                    dst = qkTr[:, 8 * p : 8 * p + 8, :, 64 * g : 64 * g + 48]
                    src_t = ptr[:].rearrange(
                        "p (s b c) -> p s b c", s=8, b=2, c=64
                    )[:, :, :, 0:48]
                    nc.scalar.copy(dst, src_t)
                # gram for this pair (needs q and k transposes of pair p);
                # fp8 DoubleRow: two subtiles per matmul
                qkTs = qkT[:].rearrange("p (s b) -> p s b", s=128, b=256)
                for j in range(0, 8, 2):
                    s = 8 * p + j
                    for h in range(2):
                        lhs = qkTs[:, s : s + 2, 128 * h : 128 * h + 128]
                        nc.tensor.matmul(
                            gram[:, 128 * h : 128 * h + 128],
                            lhs, lhs,
                            start=(s == 0), stop=(s == 126),
                            perf_mode=mybir.MatmulPerfMode.DoubleRow,
                            skip_group_check=True,
                        )

            padvfl = padvf[:]
            SP = VSPAN * RSV

            chain_accs = {}

            def chain_part(row0, nrows, dys):
                spn = nrows * RSV

                def voff(dy, dx):
                    return (row0 + 1 + dy) * RSV + 2 + dx
                slab = vdw[:, row0 * RSV : row0 * RSV + spn]
                for dy in dys:
                    c = dy + 1
                    if c == 0:
                        dst = slab
                    else:
                        acc = accp.tile([128, spn], BF16, tag=f"acc{c}",
                                        name=f"acc{c}_{row0}")
                        chain_accs[(row0, c)] = acc
                        dst = acc[:]
                    if c == 1:
                        # init on ScalarE (per-partition scaled copy of the
                        # dx=-1 tap), remaining two taps as DVE STTs
                        t = 18 + 3 * (dy + 1) + 0
                        fl = padvfl[:, voff(dy, -1) : voff(dy, -1) + spn]
                        nc.scalar.activation(
                            dst, fl, mybir.ActivationFunctionType.Copy,
                            scale=wt[:, t : t + 1],
                        )
                        rest = (0, 1)
                    else:
                        t = 18 + 3 * (dy + 1) + 1
                        fl = padvfl[:, voff(dy, 0) : voff(dy, 0) + spn]
                        nc.vector.tensor_scalar_mul(dst, fl, wt[:, t : t + 1])
                        rest = (-1, 1)
                    for dx in rest:
                        t = 18 + 3 * (dy + 1) + dx + 1
                        fl = padvfl[:, voff(dy, dx) : voff(dy, dx) + spn]
                        nc.vector.scalar_tensor_tensor(
                            dst, fl, wt[:, t : t + 1], dst,
                            op0=mybir.AluOpType.mult,
                            op1=mybir.AluOpType.add,
                        )
                if 1 in dys:
                    slabv = vdw[:, row0 * RSV : row0 * RSV + spn]
                    nc.vector.tensor_add(slabv, slabv,
                                         chain_accs[(row0, 1)][:])
                    nc.vector.tensor_add(slabv, slabv,
                                         chain_accs[(row0, 2)][:])

            for p in range(NPAIR):
                sweep_a(p)
                if p >= 1:
                    taps_pair(p - 1)
                if p == 14:
                    chain_part(96, 16, (1,))       # chain_b span 6
                    chain_part(112, 8, (-1, 0))    # chain_a half-span 7a
                elif p == 15:
                    chain_part(112, 8, (1,))       # chain_b 7a
                    chain_part(120, 8, (-1, 0))    # chain_a 7b
                elif p % 2 == 1:
                    chain_part(16 * (p // 2), 16, (-1, 0))
                elif p >= 2:
                    chain_part(16 * (p // 2 - 1), 16, (1,))
            taps_pair(NPAIR - 1)

            # ================= attention finalize ===========================
            # (emitted before the last chain part so mt is ready while the
            # final v-chain drains on DVE; both heads batched on stacked
            # partitions 0:48 / 64:112 to halve the serial op count)
            n2b = sp.tile([128, 2], F32, tag="n2b")
            for h in range(2):
                n2full = sp.tile([128, 128], F32, tag="n2full",
                                 name=f"n2full_{h}")
                nc.vector.tensor_mul(n2full[:], gram[:, 128 * h : 128 * h + 128],
                                     idf[:])
                nc.vector.reduce_sum(
                    n2b[:, h : h + 1], n2full[:], axis=mybir.AxisListType.X
                )
            nrmb = sp.tile([128, 2], F32, tag="nrmb")
            nc.scalar.sqrt(nrmb[:], n2b[:])
            nc.vector.tensor_scalar_max(nrmb[:], nrmb[:], EPS)
            rrb = sp.tile([128, 2], F32, tag="rrb")
            nc.vector.reciprocal(rrb[:], nrmb[:])

            gkq2 = sp.tile([112, 48], F32, tag="gkq2")
            nc.scalar.copy(gkq2[0:48, :], gram[64:112, 0:48])
            nc.scalar.copy(gkq2[64:112, :], gram[64:112, 128:176])
            rkb = sp.tile([112, 1], F32, tag="rkb")
            nc.vector.tensor_copy(rkb[0:48, :], rrb[64:112, 0:1])
            nc.vector.tensor_copy(rkb[64:112, :], rrb[64:112, 1:2])
            rqs = sp.tile([112, 1], F32, tag="rqs")
            nc.vector.tensor_copy(rqs[0:48, :], rrb[0:48, 0:1])
            nc.vector.tensor_copy(rqs[64:112, :], rrb[0:48, 1:2])
            askq2 = sp.tile([112, 48], F32, tag="askq2")
            nc.vector.tensor_scalar_mul(askq2[:], gkq2[:], rkb[:])
            gs2 = sp.tile([112, 48], F32, tag="gs2")
            ptr2a = pst.tile([48, 48], F32, tag="ptr", name="ptr2a")
            nc.tensor.transpose(ptr2a[:], askq2[0:48, :], idf[0:48, 0:48])
            nc.vector.tensor_copy(gs2[0:48, :], ptr2a[:])
            ptr2b = pst.tile([48, 48], F32, tag="ptr", name="ptr2b")
            nc.tensor.transpose(ptr2b[:], askq2[64:112, :],
                                idf[64:112, 64:112])
            nc.vector.tensor_copy(gs2[64:112, :], ptr2b[:])
            scb = sp.tile([112, 1], F32, tag="scb")
            nc.vector.tensor_mul(scb[:], rqs[:], tvb[:])
            mx2 = sp.tile([112, 1], F32, tag="mx2")
            nc.vector.reduce_max(mx2[:], gs2[:], axis=mybir.AxisListType.X)
            nb2 = sp.tile([112, 1], F32, tag="nb2")
            nc.vector.tensor_mul(nb2[:], mx2[:], scb[:])
            nc.vector.tensor_scalar_mul(nb2[:], nb2[:], -1.0)
            ex2 = sp.tile([112, 48], F32, tag="ex2")
            nc.scalar.activation(
                ex2[:], gs2[:], mybir.ActivationFunctionType.Exp,
                bias=nb2[:], scale=scb[:],
            )
            sm2 = sp.tile([112, 1], F32, tag="sm2")
            nc.vector.reduce_sum(sm2[:], ex2[:], axis=mybir.AxisListType.X)
            rs2 = sp.tile([112, 1], F32, tag="rs2")
            nc.vector.reciprocal(rs2[:], sm2[:])
            ab2 = sp.tile([112, 48], BF16, tag="ab2")
            nc.vector.tensor_scalar_mul(ab2[:], ex2[:], rs2[:])
            for h in range(2):
                pmt = pst.tile([48, DIM], F32, tag="ptr", name=f"pmt_{h}")
                nc.tensor.matmul(
                    pmt[:], ab2[64 * h : 64 * h + 48, :],
                    wpb[64 * h : 64 * h + 48, :], start=True, stop=True,
                )
                nc.vector.tensor_copy(mt[64 * h : 64 * h + 48, :], pmt[:])

            chain_part(120, 8, (1,))

            # ================= tail: final GEMM + output ====================
            for mj in range(2):
              for q in range(NVSP):
                accr = vdw[:, SP * q : SP * (q + 1)].rearrange(
                    "p (r x) -> p r x", r=VSPAN, x=RSV
                )
                for p2 in range(VSPAN // 8):
                    if True:
                        pout = psb.tile([GCH, PCH], F32, tag="big",
                                        name=f"pout_{q}_{p2}_{mj}")
                        for i2 in range(2):
                            rows = 8 * p2 + 4 * i2
                            nc.tensor.matmul(
                                pout[:, 512 * i2 : 512 * i2 + 512],
                                mt[:, 96 * mj : 96 * mj + 96],
                                accr[:, rows : rows + 4, 0:128],
                                start=True, stop=True,
                                skip_group_check=True,
                            )
                        ost = ostp.tile([GCH, PCH], BF16, tag="ost",
                                        name=f"ost_{q}_{p2}_{mj}")
                        if mj == 0 and q < 4:
                            # DVE is still draining v-chains when the tail
                            # starts; keep early slot-recycling on ScalarE
                            nc.scalar.copy(ost[:], pout[:])
                        elif (q + p2 + mj) % 2 == 0:
                            nc.scalar.copy(ost[:], pout[:])
                        else:
                            nc.vector.tensor_copy(ost[:], pout[:])
                        col0 = 128 * VSPAN * q + PCH * p2
                        nc.sync.dma_start(
                            outp[96 * mj : 96 * mj + 96, col0 : col0 + PCH],
                            ost[:],
                        )

    return nc


_NC_CACHE = None


def _get_nc(split=True):
    global _NC_CACHE
    if _NC_CACHE is None:
        _NC_CACHE = _build_kernel()
        if split:
            # needed for walrus codegen in this env; breaks CoreSim, so only
            # applied on the hardware path
            _split_multiwait(_NC_CACHE)
    return _NC_CACHE


def make_in_maps(x, w_qkv, w_dw, w_proj, temperature):
    x = np.asarray(x, dtype=np.float32)
    w_qkv = np.asarray(w_qkv, dtype=np.float32)
    w_dw = np.asarray(w_dw, dtype=np.float32).reshape(3 * DIM, 3, 3)
    w_proj = np.asarray(w_proj, dtype=np.float32)
    temperature = np.asarray(temperature, dtype=np.float32).reshape(HEADS)
    bf = ml_dtypes.bfloat16
    f8 = ml_dtypes.float8_e4m3fn

    in_maps = []
    for m in range(8):
        b, p = divmod(m, 2)
        rows = np.concatenate(
            [np.arange(96 * p + off, 96 * p + off + 96) for off in (0, DIM, 2 * DIM)]
        )  # q(96), k(96), v(96) global rows in w_qkv / w_dw
        wq = w_qkv[rows, :]                      # [288, 192] (q, k, v)
        dw = w_dw[rows]                          # [288, 3, 3]

        # wqT [192, 384]: 3 groups x 128 cols, heads at +0 and +64
        wqT = np.zeros((DIM, 384), dtype=np.float32)
        for g in range(3):
            wqT[:, 128 * g : 128 * g + 48] = wq[96 * g : 96 * g + 48].T
            wqT[:, 128 * g + 64 : 128 * g + 112] = wq[96 * g + 48 : 96 * g + 96].T

        # wd8 [128, 2304] fp8: per q/k group: 3 DoubleRow diag pairs
        # (dy=-1 & dy=0 for dx=-1,0,1) then 3 singles (dy=+1)
        wd8 = np.zeros((128, 2304), dtype=np.float32)
        for g in range(2):
            d = dw[96 * g : 96 * g + 96]  # [96, 3, 3]
            for ci, dx in enumerate((-1, 0, 1)):
                blk = wd8[:, 1152 * g + 256 * ci : 1152 * g + 256 * ci + 256]
                for t, dy in enumerate((-1, 0)):
                    sub = blk[:, 128 * t : 128 * t + 128]
                    np.fill_diagonal(sub[0:48, 0:48], d[0:48, dy + 1, dx + 1])
                    np.fill_diagonal(sub[64:112, 64:112],
                                     d[48:96, dy + 1, dx + 1])
                sb = wd8[:, 1152 * g + 768 + 128 * ci :
                         1152 * g + 768 + 128 * ci + 128]
                np.fill_diagonal(sb[0:48, 0:48], d[0:48, 2, dx + 1])
                np.fill_diagonal(sb[64:112, 64:112], d[48:96, 2, dx + 1])

        wpT = np.ascontiguousarray(w_proj[:, 96 * p : 96 * p + 96].T)  # [96, 192]
        wtapm = np.zeros((128, 27), dtype=np.float32)
        for g in range(3):
            for t in range(9):
                dy, dx = divmod(t, 3)
                d = dw[96 * g : 96 * g + 96, dy, dx]
                wtapm[0:48, 9 * g + t] = d[0:48]
                wtapm[64:112, 9 * g + t] = d[48:96]
        tempvm = np.empty((48, 2), dtype=np.float32)
        tempvm[:, 0] = temperature[2 * p]
        tempvm[:, 1] = temperature[2 * p + 1]
        xb = x[b].reshape(DIM, NPIX)
        in_maps.append(
            {
                "x0": xb[:96].astype(bf),
                "x1": xb[96:].astype(bf),
                "wqT0": wqT[:96].astype(bf),
                "wqT1": wqT[96:].astype(bf),
                "wd8": wd8.astype(f8),
                "wpT": wpT.astype(bf),
                "wtap": wtapm,
                "tempv": tempvm,
                "id128": np.eye(128, dtype=np.float32),
            }
        )
    return in_maps


def kernel(x, w_qkv, w_dw, w_proj, temperature):
    nc = _get_nc()
    in_maps = make_in_maps(x, w_qkv, w_dw, w_proj, temperature)
    res = run_bass_kernel_spmd(nc, in_maps, core_ids=list(range(8)))
    out = np.empty((B, DIM, HH, WW), dtype=np.float32)
    for b in range(B):
        part = (res.results[2 * b]["outp"].astype(np.float32)
                + res.results[2 * b + 1]["outp"].astype(np.float32))
        out[b] = part.reshape(DIM, HH, WW)
    return out


# revision 28
# speedup vs baseline: 1.0397x; 1.0397x over previous
"""Trainium2 Bass kernel for channel attention (XCA-style) module.

Computation (per batch b):
  qkv = w_qkv @ x          (1x1 conv, 192 -> 576 ch)
  qkv = dwconv3x3(qkv)     (depthwise, pad 1)
  q,k,v = split; per head (48 ch): l2-normalize q,k along spatial,
  attn = softmax(temp * q_hat k_hat^T); out = attn @ v
  out = w_proj @ out       (1x1 conv)

Sharding: 8 cores = 4 batches x 2 head-pairs. Each core handles one batch and
2 of the 4 heads (288 of 576 qkv channels), producing a partial projection
output [192, 16384] in bf16; host sums the two partials per batch.

v2 structure (per core):
  Phase 1 (sweep A): x streamed once; per 1024-px chunk-pair, GEMMs for all
    3 groups into [128,1024] 2-bank PSUM tiles; paired evacuation to padded
    image buffers: q/k in fp8e4 (row stride 144), v in bf16 (row stride 132).
  Phase 2 (q,k): depthwise taps fully on PE as fp8 matmuls; the (dy=-1,dy=0)
    taps per dx are fused into one DoubleRow pair matmul (2 taps/instruction);
    dy=+1 taps are plain fp8 matmuls. Evac -> bf16 -> PE transpose -> qkT
    store (DVE 2x copy) -> per-head gram accumulation.
  Phase 3 (v): depthwise taps fully on DVE as 3 accumulation chains over
    16-row flat spans (chain starts are 4x-mode tensor_scalar on the aligned
    dx=0 taps); two tensor_tensor merges; the final projection GEMM reads the
    merged accumulator directly (no copy), output evacuated as bf16.
"""

import sys

sys.path.insert(0, "/opt/trn_rl_repo")

import numpy as np
import ml_dtypes

import concourse.bass as bass
import concourse.mybir as mybir
from concourse import tile
from concourse.bass_utils import run_bass_kernel_spmd

F32 = mybir.dt.float32
BF16 = mybir.dt.bfloat16
F8 = mybir.dt.float8e4

DIM = 192
HEADS = 4
B = 4
HH = 128
WW = 128
NPIX = HH * WW          # 16384
GCH = 96                # q/k/v channels per core (2 heads x 48)
NPAIR = 16              # 1024-px chunk pairs
PCH = 1024              # pair chunk size (8 image rows)
RS8 = 144               # fp8 pad row stride (8 pad cols each side)
C08 = 8                 # fp8 pad interior col offset
RSV = 132               # bf16 v pad row stride (2 pad cols each side)
VSPAN = 16              # v-tap span in image rows (4 chunks of 512)
NVSP = HH // VSPAN      # 8 spans
EPS = 1e-12


def _split_multiwait(nc):
    """walrus in this env only encodes one sem-wait per instruction; hoist
    extra waits into single-wait NoOps placed just before the instruction."""
    for f in nc.m.functions:
        for bb in f.blocks:
            insts = bb.instructions
            i = 0
            while i < len(insts):
                inst = insts[i]
                si = getattr(inst, "sync_info", None)
                ow = list(si.on_wait) if (si is not None and si.on_wait) else []
                if len(ow) > 1:
                    nops = []
                    for w in ow[:-1]:
                        nops.append(
                            mybir.InstNoOp(
                                name=nc.get_next_instruction_name(),
                                sync_info=mybir.SyncInfo(on_wait=[w], on_update=[]),
                                bass_nofuse=True,
                                engine=inst.engine,
                            )
                        )
                    inst.sync_info = mybir.SyncInfo(
                        on_wait=[ow[-1]], on_update=list(si.on_update)
                    )
                    insts[i:i] = nops
                    i += len(nops)
                i += 1


def _pair_window(pad8, nrows_buf, row0, c0, w=128):
    """Overlapping [128, 2, 4, w] AP over an fp8 pad: two 4-row windows at
    buffer rows row0 and row0+1 (the dy=-1 / dy=0 tap pair), cols c0..c0+w.
    Pair stride RS8 satisfies the DoubleRow step%16==0 constraint."""
    r = pad8.rearrange("p (r x) -> p r x", r=nrows_buf, x=RS8)
    base = r[:, row0 : row0 + 5, c0 : c0 + w]          # [p, 5, w]
    u = base.unsqueeze(1).broadcast_to((128, 2, 5, w))  # [p, 2, 5, w] stride 0
    apv = u.ap
    apv[1] = [RS8, 2]
    c = u.copy()
    c.ap = apv
    return c[:, :, 0:4, :]


def _build_kernel():
    nc = bass.Bass("TRN2", target_bir_lowering=False, debug=False, num_devices=8)

    # ---- DRAM I/O ----
    x_half = [
        nc.dram_tensor(f"x{h}", [GCH, NPIX], BF16, kind="ExternalInput")
        for h in range(2)
    ]  # input channels 0:96 / 96:192, bf16
    # w_qkv^T column layout: 3 groups x 128-padded [h0(48) pad h1(48) pad]
    wqT0 = nc.dram_tensor("wqT0", [GCH, 384], BF16, kind="ExternalInput")
    wqT1 = nc.dram_tensor("wqT1", [GCH, 384], BF16, kind="ExternalInput")
    # fp8 diag tap weights for q/k: per group, 3 DoubleRow pairs (dy=-1,0) at
    # dx=-1,0,1 then 3 singles (dy=+1): [128, 2*(3*256+3*128)] = [128, 2304]
    wd8 = nc.dram_tensor("wd8", [128, 2304], F8, kind="ExternalInput")
    wpT = nc.dram_tensor("wpT", [GCH, DIM], BF16, kind="ExternalInput")
    wtap = nc.dram_tensor("wtap", [128, 27], F32, kind="ExternalInput")
    tempv = nc.dram_tensor("tempv", [48, 2], F32, kind="ExternalInput")
    id128 = nc.dram_tensor("id128", [128, 128], F32, kind="ExternalInput")
    outp = nc.dram_tensor("outp", [DIM, NPIX], BF16, kind="ExternalOutput")

    GLO = [0, 128, 256]    # group column offsets in wqT
    W = 128                # padded group width (h0 at 0:48, h1 at 64:112)

    with tile.TileContext(nc) as tc:
        with (
            tc.tile_pool(name="persist", bufs=1) as pp,
            tc.tile_pool(name="scratch", bufs=2) as sp,
            tc.tile_pool(name="dsc", bufs=3) as dscp,
            tc.tile_pool(name="ost", bufs=6) as ostp,
            tc.tile_pool(name="acc", bufs=2) as accp,
            tc.tile_pool(name="xstream", bufs=4) as xsp,
            tc.tile_pool(name="ps_big", bufs=3, space="PSUM") as psb,
            tc.tile_pool(name="ps_tr", bufs=1, space="PSUM") as pst,
            tc.tile_pool(name="ps_gram", bufs=1, space="PSUM") as psg,
        ):
            # ---- persistent SBUF ----
            w0 = pp.tile([GCH, 384], BF16, tag="w0")
            w1 = pp.tile([GCH, 384], BF16, tag="w1")
            nc.sync.dma_start(w0[:], wqT0[:])
            nc.sync.dma_start(w1[:], wqT1[:])
            xpre = []
            for pp_i in range(3):
                xa = xsp.tile([GCH, PCH], BF16, tag="xs0", name=f"xpre0_{pp_i}")
                xb = xsp.tile([GCH, PCH], BF16, tag="xs1", name=f"xpre1_{pp_i}")
                nc.sync.dma_start(xa[:], x_half[0][:, PCH * pp_i : PCH * (pp_i + 1)])
                nc.sync.dma_start(xb[:], x_half[1][:, PCH * pp_i : PCH * (pp_i + 1)])
                xpre.append((xa, xb))
            wd8t = pp.tile([128, 2304], F8, tag="wd8t")
            nc.sync.dma_start(wd8t[:], wd8[:])
            wpb = pp.tile([112, DIM], BF16, tag="wpb")
            for h in range(2):
                nc.sync.dma_start(wpb[64 * h : 64 * h + 48, :],
                                  wpT[48 * h : 48 * h + 48, :])
            wt = pp.tile([128, 27], F32, tag="wt")
            nc.sync.dma_start(wt[:], wtap[:])
            tv = pp.tile([48, 2], F32, tag="tv")
            nc.sync.dma_start(tv[:], tempv[:])
            tvb = pp.tile([112, 1], F32, tag="tvb")
            nc.vector.tensor_copy(tvb[0:48, :], tv[:, 0:1])
            nc.vector.tensor_copy(tvb[64:112, :], tv[:, 1:2])
            idf = pp.tile([128, 128], F32, tag="idf")
            nc.sync.dma_start(idf[:], id128[:])
            idb = pp.tile([128, 128], BF16, tag="idb")
            nc.vector.tensor_copy(idb[:], idf[:])
            # preload ACT table sets (exp, sqrt) so attn finalize pays no
            # table-load latency on the critical path
            warm = pp.tile([1, 1], F32, tag="warm")
            nc.scalar.activation(warm[:], idf[0:1, 0:1],
                                 mybir.ActivationFunctionType.Exp)
            nc.scalar.sqrt(warm[:], idf[0:1, 0:1])

            # fp8 padded image buffers for q/k: 130 rows x RS8 cols
            pad8s = []
            for gi in range(2):
                p8 = pp.tile([128, 130 * RS8], F8, tag=f"pad8_{gi}",
                             name=f"pad8_{gi}")
                r = p8[:].rearrange("p (r x) -> p r x", r=130, x=RS8)
                nc.vector.memset(r[:, 0, :], 0.0)
                nc.vector.memset(r[:, 129, :], 0.0)
                nc.scalar.memzero(r[:, :, 0:C08])
                nc.scalar.memzero(r[:, :, C08 + 128 :])
                pad8s.append(p8)

            # bf16 padded image buffer for v: 130 rows x RSV cols + tail
            padvf = pp.tile([128, 130 * RSV + 4], BF16, tag="padv")
            padv = padvf[:, 0 : 130 * RSV].rearrange(
                "p (r x) -> p r x", r=130, x=RSV
            )
            nc.vector.memset(padv[:, 0, :], 0.0)
            nc.vector.memset(padv[:, 129, :], 0.0)
            nc.vector.memset(padv[:, :, 0:2], 0.0)
            nc.vector.memset(padv[:, :, 130:132], 0.0)
            nc.vector.memset(padvf[:, 130 * RSV :], 0.0)

            # transposed q/k store: per 128-pixel subtile s, 256 columns:
            # [q_h0(48) pad(16) k_h0(48) pad(16) q_h1(48) pad(16) k_h1(48) pad(16)]
            qkT = pp.tile([128, 128 * 256], F8, tag="qkT")
            qkTr = qkT[:].rearrange("p (s b c) -> p s b c", s=128, b=2, c=128)
            qkTp = qkT[:].rearrange("p (s b c) -> p s b c", s=128, b=4, c=64)
            nc.vector.memset(qkTp[:, :, :, 48:64], 0.0)

            # gram accumulators: both heads in one PSUM bank, [128, 2*128]
            gram = psg.tile([128, 256], F32, tag="gram")

            # M^T for final GEMM: [128(d-padded), 192(o)] bf16
            mt = pp.tile([128, DIM], BF16, tag="mt")
            nc.vector.memset(mt[:], 0.0)

            # persistent v-dwconv result slab (holds all spans until mt ready)
            vdw = pp.tile([128, NVSP * VSPAN * RSV], BF16, tag="vdw")

            # ========== fused pipeline: sweepA(p) | taps(p-1) | chains ==========
            def sweep_a(p):
                if p < 3:
                    xt0, xt1 = xpre[p]
                else:
                    xt0 = xsp.tile([GCH, PCH], BF16, tag="xs0")
                    xt1 = xsp.tile([GCH, PCH], BF16, tag="xs1")
                    nc.sync.dma_start(xt0[:], x_half[0][:, PCH * p : PCH * (p + 1)])
                    nc.sync.dma_start(xt1[:], x_half[1][:, PCH * p : PCH * (p + 1)])
                for g in range(3):
                    praw = psb.tile([128, PCH], F32, tag="big",
                                    name=f"praw_{p}_{g}")
                    for h, (wh, xt) in enumerate(((w0, xt0), (w1, xt1))):
                        for hf in range(2):
                            nc.tensor.matmul(
                                praw[:, 512 * hf : 512 * hf + 512],
                                wh[:, GLO[g] : GLO[g] + W],
                                xt[:, 512 * hf : 512 * hf + 512],
                                start=(h == 0), stop=(h == 1),
                                skip_group_check=True,
                            )
                    src_v = praw[:].rearrange("p (r x) -> p r x", r=8, x=128)
                    if g < 2:
                        dst = pad8s[g][:].rearrange(
                            "p (r x) -> p r x", r=130, x=RS8
                        )[:, 8 * p + 1 : 8 * p + 9, C08 : C08 + 128]
                        nc.scalar.copy(dst, src_v)
                    else:
                        dst = padv[:, 8 * p + 1 : 8 * p + 9, 2:130]
                        if p < 2:
                            nc.vector.tensor_copy(dst, src_v)
                        else:
                            nc.scalar.copy(dst, src_v)

            def taps_pair(p):
                for g in range(2):
                    pad8 = pad8s[g][:]
                    pdw = psb.tile([128, PCH], F32, tag="big",
                                   name=f"pdw_{g}_{p}")
                    for ci, dx in enumerate((-1, 0, 1)):
                        wpair = wd8t[:, 1152 * g + 256 * ci :
                                     1152 * g + 256 * ci + 256].rearrange(
                            "p (t x) -> p t x", t=2, x=128
                        )
                        for hf in range(2):
                            r0 = 8 * p + 4 * hf
                            rhs = _pair_window(pad8, 130, r0, C08 + dx)
                            nc.tensor.matmul(
                                pdw[:, 512 * hf : 512 * hf + 512],
                                wpair, rhs,
                                start=(ci == 0), stop=False,
                                perf_mode=mybir.MatmulPerfMode.DoubleRow,
                                skip_group_check=True,
                            )
                    r3 = pad8.rearrange("p (r x) -> p r x", r=130, x=RS8)
                    for ci, dx in enumerate((-1, 0, 1)):
                        ws = wd8t[:, 1152 * g + 768 + 128 * ci :
                                  1152 * g + 768 + 128 * ci + 128]
                        for hf in range(2):
                            r0 = 8 * p + 4 * hf
                            rhs = r3[:, r0 + 2 : r0 + 6,
                                     C08 + dx : C08 + dx + 128]
                            nc.tensor.matmul(
                                pdw[:, 512 * hf : 512 * hf + 512],
                                ws, rhs,
                                start=False, stop=(ci == 2),
                                skip_group_check=True,
                            )
                    dsc = dscp.tile([128, PCH], BF16, tag="dsc",
                                    name=f"dsc_{g}_{p}")
                    nc.scalar.copy(dsc[:], pdw[:])
                    ptr = pst.tile([128, PCH], BF16, tag="ptr",
                                   name=f"ptr_{g}_{p}")
                    for j in range(8):
                        nc.tensor.transpose(
                            ptr[:, 128 * j : 128 * (j + 1)],
                            dsc[:, 128 * j : 128 * (j + 1)],
                            idb[:],
                        )
                    dst = qkTr[:, 8 * p : 8 * p + 8, :, 64 * g : 64 * g + 48]
                    src_t = ptr[:].rearrange(
                        "p (s b c) -> p s b c", s=8, b=2, c=64
                    )[:, :, :, 0:48]
                    nc.scalar.copy(dst, src_t)


            padvfl = padvf[:]
            SP = VSPAN * RSV

            chain_accs = {}

            def chain_part(q, dys):
                def voff(dy, dx):
                    return (VSPAN * q + 1 + dy) * RSV + 2 + dx
                slab = vdw[:, SP * q : SP * (q + 1)]
                for dy in dys:
                    c = dy + 1
                    if c == 0:
                        dst = slab
                    else:
                        acc = accp.tile([128, SP], BF16, tag=f"acc{c}",
                                        name=f"acc{c}_{q}")
                        chain_accs[(q, c)] = acc
                        dst = acc[:]
                    if c == 1:
                        # init on ScalarE (per-partition scaled copy of the
                        # dx=-1 tap), remaining two taps as DVE STTs
                        t = 18 + 3 * (dy + 1) + 0
                        fl = padvfl[:, voff(dy, -1) : voff(dy, -1) + SP]
                        nc.scalar.activation(
                            dst, fl, mybir.ActivationFunctionType.Copy,
                            scale=wt[:, t : t + 1],
                        )
                        rest = (0, 1)
                    else:
                        t = 18 + 3 * (dy + 1) + 1
                        fl = padvfl[:, voff(dy, 0) : voff(dy, 0) + SP]
                        nc.vector.tensor_scalar_mul(dst, fl, wt[:, t : t + 1])
                        rest = (-1, 1)
                    for dx in rest:
                        t = 18 + 3 * (dy + 1) + dx + 1
                        fl = padvfl[:, voff(dy, dx) : voff(dy, dx) + SP]
                        nc.vector.scalar_tensor_tensor(
                            dst, fl, wt[:, t : t + 1], dst,
                            op0=mybir.AluOpType.mult,
                            op1=mybir.AluOpType.add,
                        )
                if 1 in dys:
                    slabv = vdw[:, SP * q : SP * (q + 1)]
                    nc.vector.tensor_add(slabv, slabv, chain_accs[(q, 1)][:])
                    nc.vector.tensor_add(slabv, slabv, chain_accs[(q, 2)][:])

            for p in range(NPAIR):
                sweep_a(p)
                if p >= 1:
                    taps_pair(p - 1)
                if p % 2 == 1:
                    chain_part(p // 2, (-1, 0))
                elif p >= 2:
                    chain_part(p // 2 - 1, (1,))
            taps_pair(NPAIR - 1)

            # deferred gram: runs in the PE-idle window while the last
            # v-chains drain on DVE (no consumer until attn finalize)
            qkTs = qkT[:].rearrange("p (s b) -> p s b", s=128, b=256)
            for s in range(0, 128, 2):
                for h in range(2):
                    lhs = qkTs[:, s : s + 2, 128 * h : 128 * h + 128]
                    nc.tensor.matmul(
                        gram[:, 128 * h : 128 * h + 128],
                        lhs, lhs,
                        start=(s == 0), stop=(s == 126),
                        perf_mode=mybir.MatmulPerfMode.DoubleRow,
                        skip_group_check=True,
                    )

            # ================= attention finalize ===========================
            # (emitted before the last chain part so mt is ready while the
            # final v-chain drains on DVE; both heads batched on stacked
            # partitions 0:48 / 64:112 to halve the serial op count)
            n2b = sp.tile([128, 2], F32, tag="n2b")
            for h in range(2):
                n2full = sp.tile([128, 128], F32, tag="n2full",
                                 name=f"n2full_{h}")
                nc.vector.tensor_mul(n2full[:], gram[:, 128 * h : 128 * h + 128],
                                     idf[:])
                nc.vector.reduce_sum(
                    n2b[:, h : h + 1], n2full[:], axis=mybir.AxisListType.X
                )
            nrmb = sp.tile([128, 2], F32, tag="nrmb")
            nc.scalar.sqrt(nrmb[:], n2b[:])
            nc.vector.tensor_scalar_max(nrmb[:], nrmb[:], EPS)
            rrb = sp.tile([128, 2], F32, tag="rrb")
            nc.vector.reciprocal(rrb[:], nrmb[:])

            gkq2 = sp.tile([112, 48], F32, tag="gkq2")
            nc.vector.tensor_copy(gkq2[0:48, :], gram[64:112, 0:48])
            nc.vector.tensor_copy(gkq2[64:112, :], gram[64:112, 128:176])
            rkb = sp.tile([112, 1], F32, tag="rkb")
            nc.vector.tensor_copy(rkb[0:48, :], rrb[64:112, 0:1])
            nc.vector.tensor_copy(rkb[64:112, :], rrb[64:112, 1:2])
            rqs = sp.tile([112, 1], F32, tag="rqs")
            nc.vector.tensor_copy(rqs[0:48, :], rrb[0:48, 0:1])
            nc.vector.tensor_copy(rqs[64:112, :], rrb[0:48, 1:2])
            askq2 = sp.tile([112, 48], F32, tag="askq2")
            nc.vector.tensor_scalar_mul(askq2[:], gkq2[:], rkb[:])
            gs2 = sp.tile([112, 48], F32, tag="gs2")
            ptr2a = pst.tile([48, 48], F32, tag="ptr", name="ptr2a")
            nc.tensor.transpose(ptr2a[:], askq2[0:48, :], idf[0:48, 0:48])
            nc.vector.tensor_copy(gs2[0:48, :], ptr2a[:])
            ptr2b = pst.tile([48, 48], F32, tag="ptr", name="ptr2b")
            nc.tensor.transpose(ptr2b[:], askq2[64:112, :],
                                idf[64:112, 64:112])
            nc.vector.tensor_copy(gs2[64:112, :], ptr2b[:])
            scb = sp.tile([112, 1], F32, tag="scb")
            nc.vector.tensor_mul(scb[:], rqs[:], tvb[:])
            mx2 = sp.tile([112, 1], F32, tag="mx2")
            nc.vector.reduce_max(mx2[:], gs2[:], axis=mybir.AxisListType.X)
            nb2 = sp.tile([112, 1], F32, tag="nb2")
            nc.vector.tensor_mul(nb2[:], mx2[:], scb[:])
            nc.vector.tensor_scalar_mul(nb2[:], nb2[:], -1.0)
            ex2 = sp.tile([112, 48], F32, tag="ex2")
            nc.scalar.activation(
                ex2[:], gs2[:], mybir.ActivationFunctionType.Exp,
                bias=nb2[:], scale=scb[:],
            )
            sm2 = sp.tile([112, 1], F32, tag="sm2")
            nc.vector.reduce_sum(sm2[:], ex2[:], axis=mybir.AxisListType.X)
            rs2 = sp.tile([112, 1], F32, tag="rs2")
            nc.vector.reciprocal(rs2[:], sm2[:])
            ab2 = sp.tile([112, 48], BF16, tag="ab2")
            nc.vector.tensor_scalar_mul(ab2[:], ex2[:], rs2[:])
            for h in range(2):
                pmt = pst.tile([48, DIM], F32, tag="ptr", name=f"pmt_{h}")
                nc.tensor.matmul(
                    pmt[:], ab2[64 * h : 64 * h + 48, :],
                    wpb[64 * h : 64 * h + 48, :], start=True, stop=True,
                )
                nc.vector.tensor_copy(mt[64 * h : 64 * h + 48, :], pmt[:])

            chain_part(NVSP - 1, (1,))

            # ================= tail: final GEMM + output ====================
            for mj in range(2):
              for q in range(NVSP):
                accr = vdw[:, SP * q : SP * (q + 1)].rearrange(
                    "p (r x) -> p r x", r=VSPAN, x=RSV
                )
                for p2 in range(VSPAN // 8):
                    if True:
                        pout = psb.tile([GCH, PCH], F32, tag="big",
                                        name=f"pout_{q}_{p2}_{mj}")
                        for i2 in range(2):
                            rows = 8 * p2 + 4 * i2
                            nc.tensor.matmul(
                                pout[:, 512 * i2 : 512 * i2 + 512],
                                mt[:, 96 * mj : 96 * mj + 96],
                                accr[:, rows : rows + 4, 0:128],
                                start=True, stop=True,
                                skip_group_check=True,
                            )
                        ost = ostp.tile([GCH, PCH], BF16, tag="ost",
                                        name=f"ost_{q}_{p2}_{mj}")
                        if mj == 0 and q < 4:
                            # DVE is still draining v-chains when the tail
                            # starts; keep early slot-recycling on ScalarE
                            nc.scalar.copy(ost[:], pout[:])
                        elif (q + p2 + mj) % 2 == 0:
                            nc.scalar.copy(ost[:], pout[:])
                        else:
                            nc.vector.tensor_copy(ost[:], pout[:])
                        col0 = 128 * VSPAN * q + PCH * p2
                        nc.sync.dma_start(
                            outp[96 * mj : 96 * mj + 96, col0 : col0 + PCH],
                            ost[:],
                        )

    return nc


_NC_CACHE = None


def _get_nc(split=True):
    global _NC_CACHE
    if _NC_CACHE is None:
        _NC_CACHE = _build_kernel()
        if split:
            # needed for walrus codegen in this env; breaks CoreSim, so only
            # applied on the hardware path
            _split_multiwait(_NC_CACHE)
    return _NC_CACHE


def make_in_maps(x, w_qkv, w_dw, w_proj, temperature):
    x = np.asarray(x, dtype=np.float32)
    w_qkv = np.asarray(w_qkv, dtype=np.float32)
    w_dw = np.asarray(w_dw, dtype=np.float32).reshape(3 * DIM, 3, 3)
    w_proj = np.asarray(w_proj, dtype=np.float32)
    temperature = np.asarray(temperature, dtype=np.float32).reshape(HEADS)
    bf = ml_dtypes.bfloat16
    f8 = ml_dtypes.float8_e4m3fn

    in_maps = []
    for m in range(8):
        b, p = divmod(m, 2)
        rows = np.concatenate(
            [np.arange(96 * p + off, 96 * p + off + 96) for off in (0, DIM, 2 * DIM)]
        )  # q(96), k(96), v(96) global rows in w_qkv / w_dw
        wq = w_qkv[rows, :]                      # [288, 192] (q, k, v)
        dw = w_dw[rows]                          # [288, 3, 3]

        # wqT [192, 384]: 3 groups x 128 cols, heads at +0 and +64
        wqT = np.zeros((DIM, 384), dtype=np.float32)
        for g in range(3):
            wqT[:, 128 * g : 128 * g + 48] = wq[96 * g : 96 * g + 48].T
            wqT[:, 128 * g + 64 : 128 * g + 112] = wq[96 * g + 48 : 96 * g + 96].T

        # wd8 [128, 2304] fp8: per q/k group: 3 DoubleRow diag pairs
        # (dy=-1 & dy=0 for dx=-1,0,1) then 3 singles (dy=+1)
        wd8 = np.zeros((128, 2304), dtype=np.float32)
        for g in range(2):
            d = dw[96 * g : 96 * g + 96]  # [96, 3, 3]
            for ci, dx in enumerate((-1, 0, 1)):
                blk = wd8[:, 1152 * g + 256 * ci : 1152 * g + 256 * ci + 256]
                for t, dy in enumerate((-1, 0)):
                    sub = blk[:, 128 * t : 128 * t + 128]
                    np.fill_diagonal(sub[0:48, 0:48], d[0:48, dy + 1, dx + 1])
                    np.fill_diagonal(sub[64:112, 64:112],
                                     d[48:96, dy + 1, dx + 1])
                sb = wd8[:, 1152 * g + 768 + 128 * ci :
                         1152 * g + 768 + 128 * ci + 128]
                np.fill_diagonal(sb[0:48, 0:48], d[0:48, 2, dx + 1])
                np.fill_diagonal(sb[64:112, 64:112], d[48:96, 2, dx + 1])

        wpT = np.ascontiguousarray(w_proj[:, 96 * p : 96 * p + 96].T)  # [96, 192]
        wtapm = np.zeros((128, 27), dtype=np.float32)
        for g in range(3):
            for t in range(9):
                dy, dx = divmod(t, 3)
                d = dw[96 * g : 96 * g + 96, dy, dx]
                wtapm[0:48, 9 * g + t] = d[0:48]
                wtapm[64:112, 9 * g + t] = d[48:96]
        tempvm = np.empty((48, 2), dtype=np.float32)
        tempvm[:, 0] = temperature[2 * p]
        tempvm[:, 1] = temperature[2 * p + 1]
        xb = x[b].reshape(DIM, NPIX)
        in_maps.append(
            {
                "x0": xb[:96].astype(bf),
                "x1": xb[96:].astype(bf),
                "wqT0": wqT[:96].astype(bf),
                "wqT1": wqT[96:].astype(bf),
                "wd8": wd8.astype(f8),
                "wpT": wpT.astype(bf),
                "wtap": wtapm,
                "tempv": tempvm,
                "id128": np.eye(128, dtype=np.float32),
            }
        )
    return in_maps


def kernel(x, w_qkv, w_dw, w_proj, temperature):
    nc = _get_nc()
    in_maps = make_in_maps(x, w_qkv, w_dw, w_proj, temperature)
    res = run_bass_kernel_spmd(nc, in_maps, core_ids=list(range(8)))
    out = np.empty((B, DIM, HH, WW), dtype=np.float32)
    for b in range(B):
        part = (res.results[2 * b]["outp"].astype(np.float32)
                + res.results[2 * b + 1]["outp"].astype(np.float32))
        out[b] = part.reshape(DIM, HH, WW)
    return out
